# revision 18
# baseline (speedup 1.0000x reference)
import json
import sys

sys.path.insert(0, "/opt/trn_rl_repo")

import numpy as np

import concourse.bass_utils as _bu
import concourse.bass2jax as _b2j
import concourse.bass as bass
import concourse.mybir as mybir
from concourse import tile

# ---------------------------------------------------------------------------
# The walrus build in this container only supports ONE sync-wait per
# instruction; current Tile emits multi-wait instructions. Split the extra
# waits into single-wait NoOps on the same engine (engines execute their
# stream in order, so semantics are identical).
_orig_compile_bir = _bu.compile_bir_kernel


def _split_multiwaits(bir_bytes):
    d = json.loads(bir_bytes)
    n = 0
    for fn in d["functions"]:
        for blk in fn["blocks"]:
            out = []
            for ins in blk["instructions"]:
                si = ins.get("sync_info")
                waits = (si or {}).get("on_wait") or []
                if len(waits) > 1:
                    for w in waits[:-1]:
                        n += 1
                        out.append(
                            {
                                "name": f"WSPL{n}-{ins['name']}",
                                "opcode": "NoOp",
                                "engine": ins["engine"],
                                "debug": ins.get("debug", 0),
                                "ins": [],
                                "outs": [],
                                "sync_info": {"on_wait": [w]},
                            }
                        )
                    si["on_wait"] = [waits[-1]]
                out.append(ins)
            blk["instructions"] = out
    return json.dumps(d).encode()


def _patched_compile_bir(bir_json, tmpdir, neff_name="file.neff"):
    return _orig_compile_bir(_split_multiwaits(bir_json), tmpdir, neff_name)


if getattr(_bu.compile_bir_kernel, "__name__", "") != "_patched_compile_bir":
    _bu.compile_bir_kernel = _patched_compile_bir
    _b2j.compile_bir_kernel = _patched_compile_bir
# ---------------------------------------------------------------------------

# nn_MinConv2dGRUCell: x (4,32,64,32,32), h0 (4,1,64,32,32), W (128,64,3,3),
# b (128,). out = (4,32,64,32,32), h_next = out[:, -1:].
# Sharding: 8 cores = B(4) x H-half(2). Per core: conv over its 16 rows
# (+1 halo row each side), then the minGRU scan h_s = a_s*h_{s-1} + bv_s with
# a = sigmoid(-gate-b_g), bv = (1-a)*g, g = max(hidden+b_h+0.5, sigmoid(hidden+b_h)).
B, S, C_IN, H, WD = 4, 32, 64, 32, 32
HID = 64
N_CORES = 8
HH = H // 2  # 16 rows per core
PX = HH * WD  # 512 pixels per core
HPX = PX // 2  # 256 = free size after (128,256) repack
BLK = 8  # timesteps per tensor_tensor_scan instruction
NBLK = S // BLK
SEG = BLK + 1  # +1 reset column per pixel-tuple

F32 = mybir.dt.float32
F32R = mybir.dt.float32r

_CACHE = {}


def _build_nc():
    nc = bass.Bass(trn_type="TRN2")

    xpad_d = nc.dram_tensor("xpad", (S, C_IN, 18, 34), F32R, kind="ExternalInput")
    # paired taps (ky,0)+(ky,1) stacked on K; leftover taps (ky,2) separate
    wtp_d = nc.dram_tensor("wtp", (2 * C_IN, 3, 2 * HID), F32R, kind="ExternalInput")
    wts_d = nc.dram_tensor("wts", (C_IN, 3, 2 * HID), F32R, kind="ExternalInput")
    h0_d = nc.dram_tensor("h0r", (HID, PX), F32, kind="ExternalInput")
    # bias columns: 0 = -b_gate (rows 0:64), 1 = b_hid, 2 = b_hid+0.5
    # (rows 64:128), 3 = +b_gate (rows 0:64)
    bias_d = nc.dram_tensor("biases", (128, 4), F32, kind="ExternalInput")
    # raw scan-layout output: [block, channel, px*SEG] — host strips reset
    # columns and reorders; keeps the device-side DMA fully contiguous.
    out_d = nc.dram_tensor("out", (NBLK, HID, PX * SEG), F32, kind="ExternalOutput")

    AL = mybir.AluOpType
    ACTF = mybir.ActivationFunctionType

    with tile.TileContext(nc) as tc:
        with (
            tc.tile_pool(name="const", bufs=1) as constp,
            tc.tile_pool(name="xin", bufs=3) as xinp,
            tc.tile_pool(name="psum", bufs=6, space="PSUM") as psump,
            tc.tile_pool(name="ew", bufs=3) as ewp,
            tc.tile_pool(name="scan", bufs=2) as scanp,
            tc.tile_pool(name="outb", bufs=2) as outp,
        ):
            wtp_t = constp.tile([2 * C_IN, 3 * 2 * HID], F32R)
            nc.sync.dma_start(wtp_t[:], wtp_d[:, :, :])
            wts_t = constp.tile([C_IN, 3 * 2 * HID], F32R)
            nc.sync.dma_start(wts_t[:], wts_d[:, :, :])
            bias_t = constp.tile([128, 4], F32)
            nc.sync.dma_start(bias_t[:], bias_d[:, :])
            h0_t = constp.tile([HID, PX], F32)
            nc.sync.dma_start(h0_t[:], h0_d[:, :])

            o_prev = None
            for bs in range(NBLK):
                a_t = scanp.tile([HID, PX * SEG], F32, tag="a")
                bv_t = scanp.tile([HID, PX * SEG], F32, tag="bv")
                a3 = a_t[:].rearrange("p (px j) -> p px j", j=SEG)
                bv3 = bv_t[:].rearrange("p (px j) -> p px j", j=SEG)

                # reset columns: a=0, bv=h_init  =>  state restarts at h_init
                nc.gpsimd.memset(a3[:, :, 0], 0.0)
                if bs == 0:
                    nc.vector.tensor_copy(bv3[:, :, 0], h0_t[:])
                else:
                    op3 = o_prev[:].rearrange("p (px j) -> p px j", j=SEG)
                    nc.vector.tensor_copy(bv3[:, :, 0], op3[:, :, SEG - 1])

                for j in range(BLK):
                    s = bs * BLK + j
                    # x tile: partitions 0:64 = xpad, 64:128 = xpad shifted
                    # left by one column (tap kx+1 when read at kx)
                    x_t = xinp.tile([2 * C_IN, 18 * 34], F32R)
                    x3 = x_t[:].rearrange("p (r c) -> p r c", r=18)
                    nc.sync.dma_start(x3[0:C_IN, :, :], xpad_d[s, :, :, :])
                    nc.sync.dma_start(x3[C_IN:, :, 0:33], xpad_d[s, :, :, 1:34])

                    p_t = psump.tile([128, PX], F32)
                    for g in range(3):
                        # taps (g,0)+(g,1) in one K=128 matmul
                        nc.tensor.matmul(
                            p_t[:],
                            wtp_t[:, g * 128 : (g + 1) * 128],
                            x3[:, g : g + HH, 0:WD],
                            start=(g == 0),
                            stop=False,
                        )
                    for g in range(3):
                        # leftover tap (g,2), K=64
                        nc.tensor.matmul(
                            p_t[:],
                            wts_t[:, g * 128 : (g + 1) * 128],
                            x3[0:C_IN, g : g + HH, 2 : 2 + WD],
                            start=False,
                            stop=(g == 2),
                        )

                    # a = sigmoid(-gate - b_g)   [partitions 0:64]
                    nc.scalar.activation(
                        a3[:, :, 1 + j],
                        p_t[0:HID, :],
                        ACTF.Sigmoid,
                        bias=bias_t[0:HID, 0:1],
                        scale=-1.0,
                    )
                    # sig_h = sigmoid(hidden + b_h)   [partitions 64:128]
                    sh_t = ewp.tile([128, PX], F32, tag="sh")
                    nc.scalar.activation(
                        sh_t[HID:, :], p_t[HID:, :], ACTF.Sigmoid,
                        bias=bias_t[HID:, 1:2],
                    )
                    # g = max(hidden + b_h + 0.5, sig_h)   [partitions 64:128]
                    g_t = ewp.tile([128, PX], F32, tag="g")
                    nc.vector.scalar_tensor_tensor(
                        g_t[HID:, :], p_t[HID:, :], bias_t[HID:, 2:3], sh_t[HID:, :],
                        op0=AL.add, op1=AL.max,
                    )
                    # shift g down to the gate lanes
                    gl_t = ewp.tile([HID, PX], F32, tag="gl")
                    nc.sync.dma_start(gl_t[:], g_t[HID:, :])
                    # z = 1 - a: alternate engines to balance ACT vs DVE load
                    z_t = ewp.tile([HID, PX], F32, tag="z")
                    if s % 2 == 0:
                        nc.scalar.activation(
                            z_t[:], p_t[0:HID, :], ACTF.Sigmoid,
                            bias=bias_t[0:HID, 3:4],
                        )
                    else:
                        nc.vector.tensor_scalar(
                            z_t[:], a3[:, :, 1 + j], -1.0, 1.0,
                            op0=AL.mult, op1=AL.add,
                        )
                    # bv = z * g   (Pool engine, frees DVE for the scan)
                    nc.gpsimd.tensor_tensor(bv3[:, :, 1 + j], z_t[:], gl_t[:], op=AL.mult)

                o_t = outp.tile([HID, PX * SEG], F32, tag="o")
                # h = a*h + bv along the segmented free dim
                nc.vector.tensor_tensor_scan(
                    o_t[:], a_t[:], bv_t[:], 0.0, op0=AL.mult, op1=AL.add
                )
                nc.sync.dma_start(out_d[bs, :, :], o_t[:])
                o_prev = o_t

    return nc


def kernel(x, h0, W, b):
    x = np.ascontiguousarray(x, dtype=np.float32)
    h0 = np.ascontiguousarray(h0, dtype=np.float32)
    W = np.ascontiguousarray(W, dtype=np.float32)
    b = np.ascontiguousarray(b, dtype=np.float32)

    if "nc" not in _CACHE:
        _CACHE["nc"] = _build_nc()
    nc = _CACHE["nc"]

    # host-side prep shared across cores
    wt = W.transpose(1, 2, 3, 0).reshape(C_IN, 9, 2 * HID)  # (ic, ky*3+kx, oc)
    wtp = np.ascontiguousarray(
        np.concatenate([wt[:, [0, 3, 6], :], wt[:, [1, 4, 7], :]], axis=0)
    )  # (128, 3, 128): K = [tap(ky,0) ics; tap(ky,1) ics]
    wts = np.ascontiguousarray(wt[:, [2, 5, 8], :])  # (64, 3, 128)
    biases = np.zeros((128, 4), dtype=np.float32)
    biases[0:HID, 0] = -b[0:HID]
    biases[HID:, 1] = b[HID:]
    biases[HID:, 2] = b[HID:] + 0.5
    biases[0:HID, 3] = b[0:HID]

    in_maps = []
    for core in range(N_CORES):
        bi, hh = core // 2, core % 2
        xpad = np.zeros((S, C_IN, 18, 34), dtype=np.float32)
        r0 = hh * HH - 1  # global row of xpad row 0
        lo, hi = max(r0, 0), min(r0 + 18, H)
        xpad[:, :, lo - r0 : hi - r0, 1:33] = x[bi, :, :, lo:hi, :]
        h0r = np.ascontiguousarray(
            h0[bi, 0, :, hh * HH : (hh + 1) * HH, :].reshape(HID, PX)
        )
        in_maps.append(
            {"xpad": xpad, "wtp": wtp, "wts": wts, "h0r": h0r, "biases": biases}
        )

    res = _bu.run_bass_kernel_spmd(nc, in_maps, core_ids=list(range(N_CORES)))

    out = np.empty((B, S, HID, H, WD), dtype=np.float32)
    for core in range(N_CORES):
        bi, hh = core // 2, core % 2
        raw = res.results[core]["out"].reshape(NBLK, HID, PX, SEG)[:, :, :, 1:]
        # (blk, c, px, j) -> (s=blk*BLK+j, c, y, x)
        core_out = raw.transpose(0, 3, 1, 2).reshape(S, HID, HH, WD)
        out[bi, :, :, hh * HH : (hh + 1) * HH, :] = core_out
    h_next = out[:, -1:].copy()
    return out, h_next


# revision 19
# speedup vs baseline: 1.0899x; 1.0899x over previous
import json
import sys

sys.path.insert(0, "/opt/trn_rl_repo")

import numpy as np

import concourse.bass_utils as _bu
import concourse.bass2jax as _b2j
import concourse.bass as bass
import concourse.mybir as mybir
from concourse import tile

# ---------------------------------------------------------------------------
# The walrus build in this container only supports ONE sync-wait per
# instruction; current Tile emits multi-wait instructions. Split the extra
# waits into single-wait NoOps on the same engine (engines execute their
# stream in order, so semantics are identical).
_orig_compile_bir = _bu.compile_bir_kernel


def _split_multiwaits(bir_bytes):
    d = json.loads(bir_bytes)
    n = 0
    for fn in d["functions"]:
        for blk in fn["blocks"]:
            out = []
            for ins in blk["instructions"]:
                si = ins.get("sync_info")
                waits = (si or {}).get("on_wait") or []
                if len(waits) > 1:
                    for w in waits[:-1]:
                        n += 1
                        out.append(
                            {
                                "name": f"WSPL{n}-{ins['name']}",
                                "opcode": "NoOp",
                                "engine": ins["engine"],
                                "debug": ins.get("debug", 0),
                                "ins": [],
                                "outs": [],
                                "sync_info": {"on_wait": [w]},
                            }
                        )
                    si["on_wait"] = [waits[-1]]
                out.append(ins)
            blk["instructions"] = out
    return json.dumps(d).encode()


def _patched_compile_bir(bir_json, tmpdir, neff_name="file.neff"):
    return _orig_compile_bir(_split_multiwaits(bir_json), tmpdir, neff_name)


if getattr(_bu.compile_bir_kernel, "__name__", "") != "_patched_compile_bir":
    _bu.compile_bir_kernel = _patched_compile_bir
    _b2j.compile_bir_kernel = _patched_compile_bir
# ---------------------------------------------------------------------------

# nn_MinConv2dGRUCell: x (4,32,64,32,32), h0 (4,1,64,32,32), W (128,64,3,3),
# b (128,). out = (4,32,64,32,32), h_next = out[:, -1:].
# Sharding: 8 cores = B(4) x H-half(2). Per core: conv over its 16 rows
# (+1 halo row each side), then the minGRU scan h_s = a_s*h_{s-1} + bv_s with
# a = sigmoid(-gate-b_g), bv = (1-a)*g, g = max(hidden+b_h+0.5, sigmoid(hidden+b_h)).
B, S, C_IN, H, WD = 4, 32, 64, 32, 32
HID = 64
N_CORES = 8
HH = H // 2  # 16 rows per core
PX = HH * WD  # 512 pixels per core
HPX = PX // 2  # 256 = free size after (128,256) repack
BLK = 8  # timesteps per tensor_tensor_scan instruction
NBLK = S // BLK
SEG = BLK + 1  # +1 reset column per pixel-tuple

F32 = mybir.dt.float32
F32R = mybir.dt.float32r

_CACHE = {}


def _build_nc():
    nc = bass.Bass(trn_type="TRN2")

    xpad_d = nc.dram_tensor("xpad", (S, C_IN, 18, 34), F32R, kind="ExternalInput")
    # paired taps (ky,0)+(ky,1) stacked on K; leftover taps (ky,2) separate
    wtp_d = nc.dram_tensor("wtp", (2 * C_IN, 3, 2 * HID), F32R, kind="ExternalInput")
    wts_d = nc.dram_tensor("wts", (C_IN, 3, 2 * HID), F32R, kind="ExternalInput")
    h0_d = nc.dram_tensor("h0r", (HID, PX), F32, kind="ExternalInput")
    # bias columns: 0 = -b_gate (rows 0:64), 1 = b_hid, 2 = b_hid+0.5
    # (rows 64:128), 3 = +b_gate (rows 0:64)
    bias_d = nc.dram_tensor("biases", (128, 4), F32, kind="ExternalInput")
    # raw scan-layout output: [block, channel, px*SEG] — host strips reset
    # columns and reorders; keeps the device-side DMA fully contiguous.
    out_d = nc.dram_tensor("out", (NBLK, HID, PX * SEG), F32, kind="ExternalOutput")

    AL = mybir.AluOpType
    ACTF = mybir.ActivationFunctionType

    with tile.TileContext(nc) as tc:
        with (
            tc.tile_pool(name="const", bufs=1) as constp,
            tc.tile_pool(name="xin", bufs=3) as xinp,
            tc.tile_pool(name="psum", bufs=6, space="PSUM") as psump,
            tc.tile_pool(name="ew", bufs=3) as ewp,
            tc.tile_pool(name="scan", bufs=2) as scanp,
            tc.tile_pool(name="outb", bufs=2) as outp,
        ):
            wtp_t = constp.tile([2 * C_IN, 3 * 2 * HID], F32R)
            nc.sync.dma_start(wtp_t[:], wtp_d[:, :, :])
            wts_t = constp.tile([C_IN, 3 * 2 * HID], F32R)
            nc.sync.dma_start(wts_t[:], wts_d[:, :, :])
            bias_t = constp.tile([128, 4], F32)
            nc.sync.dma_start(bias_t[:], bias_d[:, :])
            h0_t = constp.tile([HID, PX], F32)
            nc.sync.dma_start(h0_t[:], h0_d[:, :])

            o_prev = None
            for bs in range(NBLK):
                a_t = scanp.tile([HID, PX * SEG], F32, tag="a")
                bv_t = scanp.tile([HID, PX * SEG], F32, tag="bv")
                a3 = a_t[:].rearrange("p (px j) -> p px j", j=SEG)
                bv3 = bv_t[:].rearrange("p (px j) -> p px j", j=SEG)

                # reset columns: a=0, bv=h_init  =>  state restarts at h_init
                nc.gpsimd.memset(a3[:, :, 0], 0.0)
                if bs == 0:
                    nc.vector.tensor_copy(bv3[:, :, 0], h0_t[:])
                else:
                    op3 = o_prev[:].rearrange("p (px j) -> p px j", j=SEG)
                    nc.vector.tensor_copy(bv3[:, :, 0], op3[:, :, SEG - 1])

                for j in range(BLK):
                    s = bs * BLK + j
                    # x tile: partitions 0:64 = xpad, 64:128 = xpad shifted
                    # left by one column (tap kx+1 when read at kx)
                    x_t = xinp.tile([2 * C_IN, 18 * 34], F32R)
                    x3 = x_t[:].rearrange("p (r c) -> p r c", r=18)
                    xflat = xpad_d[s, :, :, :].rearrange("c r w -> c (r w)")
                    nc.sync.dma_start(x3[0:C_IN, :, :], xpad_d[s, :, :, :])
                    # shifted copy: bottom[f] = xpad[f+1]; the row-boundary
                    # bleed lands in pad col 33, which no tap reads
                    nc.sync.dma_start(x_t[C_IN:, 0:611], xflat[:, 1:612])

                    p_t = psump.tile([128, PX], F32)
                    for g in range(3):
                        # taps (g,0)+(g,1) in one K=128 matmul
                        nc.tensor.matmul(
                            p_t[:],
                            wtp_t[:, g * 128 : (g + 1) * 128],
                            x3[:, g : g + HH, 0:WD],
                            start=(g == 0),
                            stop=False,
                        )
                    for g in range(3):
                        # leftover tap (g,2), K=64
                        nc.tensor.matmul(
                            p_t[:],
                            wts_t[:, g * 128 : (g + 1) * 128],
                            x3[0:C_IN, g : g + HH, 2 : 2 + WD],
                            start=False,
                            stop=(g == 2),
                        )

                    # a = sigmoid(-gate - b_g)   [partitions 0:64]
                    nc.scalar.activation(
                        a3[:, :, 1 + j],
                        p_t[0:HID, :],
                        ACTF.Sigmoid,
                        bias=bias_t[0:HID, 0:1],
                        scale=-1.0,
                    )
                    # sig_h = sigmoid(hidden + b_h)   [partitions 64:128]
                    sh_t = ewp.tile([128, PX], F32, tag="sh")
                    nc.scalar.activation(
                        sh_t[HID:, :], p_t[HID:, :], ACTF.Sigmoid,
                        bias=bias_t[HID:, 1:2],
                    )
                    # g = max(hidden + b_h + 0.5, sig_h)   [partitions 64:128]
                    g_t = ewp.tile([128, PX], F32, tag="g")
                    nc.vector.scalar_tensor_tensor(
                        g_t[HID:, :], p_t[HID:, :], bias_t[HID:, 2:3], sh_t[HID:, :],
                        op0=AL.add, op1=AL.max,
                    )
                    # shift g down to the gate lanes
                    gl_t = ewp.tile([HID, PX], F32, tag="gl")
                    nc.sync.dma_start(gl_t[:], g_t[HID:, :])
                    # z = 1 - a: alternate engines to balance ACT vs DVE load
                    z_t = ewp.tile([HID, PX], F32, tag="z")
                    if s % 2 == 0:
                        nc.scalar.activation(
                            z_t[:], p_t[0:HID, :], ACTF.Sigmoid,
                            bias=bias_t[0:HID, 3:4],
                        )
                    else:
                        nc.vector.tensor_scalar(
                            z_t[:], a3[:, :, 1 + j], -1.0, 1.0,
                            op0=AL.mult, op1=AL.add,
                        )
                    # bv = z * g   (Pool engine, frees DVE for the scan)
                    nc.gpsimd.tensor_tensor(bv3[:, :, 1 + j], z_t[:], gl_t[:], op=AL.mult)

                o_t = outp.tile([HID, PX * SEG], F32, tag="o")
                # h = a*h + bv along the segmented free dim
                nc.vector.tensor_tensor_scan(
                    o_t[:], a_t[:], bv_t[:], 0.0, op0=AL.mult, op1=AL.add
                )
                nc.sync.dma_start(out_d[bs, :, :], o_t[:])
                o_prev = o_t

    return nc


def kernel(x, h0, W, b):
    x = np.ascontiguousarray(x, dtype=np.float32)
    h0 = np.ascontiguousarray(h0, dtype=np.float32)
    W = np.ascontiguousarray(W, dtype=np.float32)
    b = np.ascontiguousarray(b, dtype=np.float32)

    if "nc" not in _CACHE:
        _CACHE["nc"] = _build_nc()
    nc = _CACHE["nc"]

    # host-side prep shared across cores
    wt = W.transpose(1, 2, 3, 0).reshape(C_IN, 9, 2 * HID)  # (ic, ky*3+kx, oc)
    wtp = np.ascontiguousarray(
        np.concatenate([wt[:, [0, 3, 6], :], wt[:, [1, 4, 7], :]], axis=0)
    )  # (128, 3, 128): K = [tap(ky,0) ics; tap(ky,1) ics]
    wts = np.ascontiguousarray(wt[:, [2, 5, 8], :])  # (64, 3, 128)
    biases = np.zeros((128, 4), dtype=np.float32)
    biases[0:HID, 0] = -b[0:HID]
    biases[HID:, 1] = b[HID:]
    biases[HID:, 2] = b[HID:] + 0.5
    biases[0:HID, 3] = b[0:HID]

    in_maps = []
    for core in range(N_CORES):
        bi, hh = core // 2, core % 2
        xpad = np.zeros((S, C_IN, 18, 34), dtype=np.float32)
        r0 = hh * HH - 1  # global row of xpad row 0
        lo, hi = max(r0, 0), min(r0 + 18, H)
        xpad[:, :, lo - r0 : hi - r0, 1:33] = x[bi, :, :, lo:hi, :]
        h0r = np.ascontiguousarray(
            h0[bi, 0, :, hh * HH : (hh + 1) * HH, :].reshape(HID, PX)
        )
        in_maps.append(
            {"xpad": xpad, "wtp": wtp, "wts": wts, "h0r": h0r, "biases": biases}
        )

    res = _bu.run_bass_kernel_spmd(nc, in_maps, core_ids=list(range(N_CORES)))

    out = np.empty((B, S, HID, H, WD), dtype=np.float32)
    for core in range(N_CORES):
        bi, hh = core // 2, core % 2
        raw = res.results[core]["out"].reshape(NBLK, HID, PX, SEG)[:, :, :, 1:]
        # (blk, c, px, j) -> (s=blk*BLK+j, c, y, x)
        core_out = raw.transpose(0, 3, 1, 2).reshape(S, HID, HH, WD)
        out[bi, :, :, hh * HH : (hh + 1) * HH, :] = core_out
    h_next = out[:, -1:].copy()
    return out, h_next


# revision 21
# speedup vs baseline: 1.1180x; 1.0258x over previous
import json
import sys

sys.path.insert(0, "/opt/trn_rl_repo")

import numpy as np

import concourse.bass_utils as _bu
import concourse.bass2jax as _b2j
import concourse.bass as bass
import concourse.mybir as mybir
from concourse import tile

# ---------------------------------------------------------------------------
# The walrus build in this container only supports ONE sync-wait per
# instruction; current Tile emits multi-wait instructions. Split the extra
# waits into single-wait NoOps on the same engine (engines execute their
# stream in order, so semantics are identical).
_orig_compile_bir = _bu.compile_bir_kernel


def _split_multiwaits(bir_bytes):
    d = json.loads(bir_bytes)
    n = 0
    for fn in d["functions"]:
        for blk in fn["blocks"]:
            out = []
            for ins in blk["instructions"]:
                si = ins.get("sync_info")
                waits = (si or {}).get("on_wait") or []
                if len(waits) > 1:
                    for w in waits[:-1]:
                        n += 1
                        out.append(
                            {
                                "name": f"WSPL{n}-{ins['name']}",
                                "opcode": "NoOp",
                                "engine": ins["engine"],
                                "debug": ins.get("debug", 0),
                                "ins": [],
                                "outs": [],
                                "sync_info": {"on_wait": [w]},
                            }
                        )
                    si["on_wait"] = [waits[-1]]
                out.append(ins)
            blk["instructions"] = out
    return json.dumps(d).encode()


def _patched_compile_bir(bir_json, tmpdir, neff_name="file.neff"):
    return _orig_compile_bir(_split_multiwaits(bir_json), tmpdir, neff_name)


if getattr(_bu.compile_bir_kernel, "__name__", "") != "_patched_compile_bir":
    _bu.compile_bir_kernel = _patched_compile_bir
    _b2j.compile_bir_kernel = _patched_compile_bir
# ---------------------------------------------------------------------------

# nn_MinConv2dGRUCell: x (4,32,64,32,32), h0 (4,1,64,32,32), W (128,64,3,3),
# b (128,). out = (4,32,64,32,32), h_next = out[:, -1:].
# Sharding: 8 cores = B(4) x H-half(2). Per core: conv over its 16 rows
# (+1 halo row each side), then the minGRU scan h_s = a_s*h_{s-1} + bv_s with
# a = sigmoid(-gate-b_g), bv = (1-a)*g, g = max(hidden+b_h+0.5, sigmoid(hidden+b_h)).
B, S, C_IN, H, WD = 4, 32, 64, 32, 32
HID = 64
N_CORES = 8
HH = H // 2  # 16 rows per core
PX = HH * WD  # 512 pixels per core
HPX = PX // 2  # 256 = free size after (128,256) repack
BLK = 8  # timesteps per tensor_tensor_scan instruction
NBLK = S // BLK
SEG = BLK + 1  # +1 reset column per pixel-tuple

F32 = mybir.dt.float32
F32R = mybir.dt.float32r
BF16 = mybir.dt.bfloat16

_CACHE = {}


def _build_nc():
    nc = bass.Bass(trn_type="TRN2")

    xpad_d = nc.dram_tensor("xpad", (S, C_IN, 18, 34), BF16, kind="ExternalInput")
    # paired taps (ky,0)+(ky,1) stacked on K; leftover taps (ky,2) separate
    wtp_d = nc.dram_tensor("wtp", (2 * C_IN, 3, 2 * HID), BF16, kind="ExternalInput")
    wts_d = nc.dram_tensor("wts", (C_IN, 3, 2 * HID), BF16, kind="ExternalInput")
    h0_d = nc.dram_tensor("h0r", (HID, PX), F32, kind="ExternalInput")
    # bias columns: 0 = -b_gate (rows 0:64), 1 = b_hid, 2 = b_hid+0.5
    # (rows 64:128), 3 = +b_gate (rows 0:64)
    bias_d = nc.dram_tensor("biases", (128, 4), F32, kind="ExternalInput")
    # raw scan-layout output: [block, channel, px*SEG] — host strips reset
    # columns and reorders; keeps the device-side DMA fully contiguous.
    out_d = nc.dram_tensor("out", (NBLK, HID, PX * SEG), F32, kind="ExternalOutput")

    AL = mybir.AluOpType
    ACTF = mybir.ActivationFunctionType

    with tile.TileContext(nc) as tc:
        with (
            tc.tile_pool(name="const", bufs=1) as constp,
            tc.tile_pool(name="xin", bufs=3) as xinp,
            tc.tile_pool(name="psum", bufs=6, space="PSUM") as psump,
            tc.tile_pool(name="ew", bufs=3) as ewp,
            tc.tile_pool(name="scan", bufs=2) as scanp,
            tc.tile_pool(name="outb", bufs=2) as outp,
        ):
            wtp_t = constp.tile([2 * C_IN, 3 * 2 * HID], BF16)
            nc.sync.dma_start(wtp_t[:], wtp_d[:, :, :])
            wts_t = constp.tile([C_IN, 3 * 2 * HID], BF16)
            nc.sync.dma_start(wts_t[:], wts_d[:, :, :])
            bias_t = constp.tile([128, 4], F32)
            nc.sync.dma_start(bias_t[:], bias_d[:, :])
            h0_t = constp.tile([HID, PX], F32)
            nc.sync.dma_start(h0_t[:], h0_d[:, :])

            o_prev = None
            for bs in range(NBLK):
                a_t = scanp.tile([HID, PX * SEG], F32, tag="a")
                bv_t = scanp.tile([HID, PX * SEG], F32, tag="bv")
                a3 = a_t[:].rearrange("p (px j) -> p px j", j=SEG)
                bv3 = bv_t[:].rearrange("p (px j) -> p px j", j=SEG)

                # reset columns: a=0, bv=h_init  =>  state restarts at h_init
                nc.gpsimd.memset(a3[:, :, 0], 0.0)
                if bs == 0:
                    nc.vector.tensor_copy(bv3[:, :, 0], h0_t[:])
                else:
                    op3 = o_prev[:].rearrange("p (px j) -> p px j", j=SEG)
                    nc.vector.tensor_copy(bv3[:, :, 0], op3[:, :, SEG - 1])

                for j in range(BLK):
                    s = bs * BLK + j
                    # x tile: partitions 0:64 = xpad, 64:128 = xpad shifted
                    # left by one column (tap kx+1 when read at kx)
                    x_t = xinp.tile([2 * C_IN, 18 * 34], BF16)
                    x3 = x_t[:].rearrange("p (r c) -> p r c", r=18)
                    xflat = xpad_d[s, :, :, :].rearrange("c r w -> c (r w)")
                    nc.scalar.dma_start(x3[0:C_IN, :, :], xpad_d[s, :, :, :])
                    # shifted copy: bottom[f] = xpad[f+1]; the row-boundary
                    # bleed lands in pad col 33, which no tap reads
                    nc.gpsimd.dma_start(x_t[C_IN:, 0:611], xflat[:, 1:612])

                    p_t = psump.tile([128, PX], F32)
                    for g in range(3):
                        # taps (g,0)+(g,1) in one K=128 matmul
                        nc.tensor.matmul(
                            p_t[:],
                            wtp_t[:, g * 128 : (g + 1) * 128],
                            x3[:, g : g + HH, 0:WD],
                            start=(g == 0),
                            stop=False,
                        )
                    for g in range(3):
                        # leftover tap (g,2), K=64
                        nc.tensor.matmul(
                            p_t[:],
                            wts_t[:, g * 128 : (g + 1) * 128],
                            x3[0:C_IN, g : g + HH, 2 : 2 + WD],
                            start=False,
                            stop=(g == 2),
                        )

                    # a = sigmoid(-gate - b_g)   [partitions 0:64]
                    nc.scalar.activation(
                        a3[:, :, 1 + j],
                        p_t[0:HID, :],
                        ACTF.Sigmoid,
                        bias=bias_t[0:HID, 0:1],
                        scale=-1.0,
                    )
                    # sig_h = sigmoid(hidden + b_h)   [partitions 64:128]
                    sh_t = ewp.tile([128, PX], F32, tag="sh")
                    nc.scalar.activation(
                        sh_t[HID:, :], p_t[HID:, :], ACTF.Sigmoid,
                        bias=bias_t[HID:, 1:2],
                    )
                    # g = max(hidden + b_h + 0.5, sig_h)   [partitions 64:128]
                    g_t = ewp.tile([128, PX], F32, tag="g")
                    nc.vector.scalar_tensor_tensor(
                        g_t[HID:, :], p_t[HID:, :], bias_t[HID:, 2:3], sh_t[HID:, :],
                        op0=AL.add, op1=AL.max,
                    )
                    # shift g down to the gate lanes
                    gl_t = ewp.tile([HID, PX], F32, tag="gl")
                    nc.scalar.dma_start(gl_t[:], g_t[HID:, :])
                    # z = 1 - a: alternate engines to balance ACT vs DVE load
                    z_t = ewp.tile([HID, PX], F32, tag="z")
                    if s % 2 == 0:
                        nc.scalar.activation(
                            z_t[:], p_t[0:HID, :], ACTF.Sigmoid,
                            bias=bias_t[0:HID, 3:4],
                        )
                    else:
                        nc.vector.tensor_scalar(
                            z_t[:], a3[:, :, 1 + j], -1.0, 1.0,
                            op0=AL.mult, op1=AL.add,
                        )
                    # bv = z * g   (Pool engine, frees DVE for the scan)
                    nc.gpsimd.tensor_tensor(bv3[:, :, 1 + j], z_t[:], gl_t[:], op=AL.mult)

                o_t = outp.tile([HID, PX * SEG], F32, tag="o")
                # h = a*h + bv along the segmented free dim
                nc.vector.tensor_tensor_scan(
                    o_t[:], a_t[:], bv_t[:], 0.0, op0=AL.mult, op1=AL.add
                )
                nc.sync.dma_start(out_d[bs, :, :], o_t[:])
                o_prev = o_t

    return nc


def kernel(x, h0, W, b):
    x = np.ascontiguousarray(x, dtype=np.float32)
    import ml_dtypes
    h0 = np.ascontiguousarray(h0, dtype=np.float32)
    W = np.ascontiguousarray(W, dtype=np.float32)
    b = np.ascontiguousarray(b, dtype=np.float32)

    if "nc" not in _CACHE:
        _CACHE["nc"] = _build_nc()
    nc = _CACHE["nc"]

    # host-side prep shared across cores
    wt = W.transpose(1, 2, 3, 0).reshape(C_IN, 9, 2 * HID)  # (ic, ky*3+kx, oc)
    wtp = np.ascontiguousarray(
        np.concatenate([wt[:, [0, 3, 6], :], wt[:, [1, 4, 7], :]], axis=0)
    ).astype(ml_dtypes.bfloat16)  # (128, 3, 128)
    wts = np.ascontiguousarray(wt[:, [2, 5, 8], :]).astype(ml_dtypes.bfloat16)
    biases = np.zeros((128, 4), dtype=np.float32)
    biases[0:HID, 0] = -b[0:HID]
    biases[HID:, 1] = b[HID:]
    biases[HID:, 2] = b[HID:] + 0.5
    biases[0:HID, 3] = b[0:HID]

    in_maps = []
    for core in range(N_CORES):
        bi, hh = core // 2, core % 2
        xpad = np.zeros((S, C_IN, 18, 34), dtype=ml_dtypes.bfloat16)
        r0 = hh * HH - 1  # global row of xpad row 0
        lo, hi = max(r0, 0), min(r0 + 18, H)
        xpad[:, :, lo - r0 : hi - r0, 1:33] = x[bi, :, :, lo:hi, :]
        h0r = np.ascontiguousarray(
            h0[bi, 0, :, hh * HH : (hh + 1) * HH, :].reshape(HID, PX)
        )
        in_maps.append(
            {"xpad": xpad, "wtp": wtp, "wts": wts, "h0r": h0r, "biases": biases}
        )

    res = _bu.run_bass_kernel_spmd(nc, in_maps, core_ids=list(range(N_CORES)))

    out = np.empty((B, S, HID, H, WD), dtype=np.float32)
    for core in range(N_CORES):
        bi, hh = core // 2, core % 2
        raw = res.results[core]["out"].reshape(NBLK, HID, PX, SEG)[:, :, :, 1:]
        # (blk, c, px, j) -> (s=blk*BLK+j, c, y, x)
        core_out = raw.transpose(0, 3, 1, 2).reshape(S, HID, HH, WD)
        out[bi, :, :, hh * HH : (hh + 1) * HH, :] = core_out
    h_next = out[:, -1:].copy()
    return out, h_next


# revision 22
# speedup vs baseline: 1.1992x; 1.0726x over previous
import json
import sys

sys.path.insert(0, "/opt/trn_rl_repo")

import numpy as np

import concourse.bass_utils as _bu
import concourse.bass2jax as _b2j
import concourse.bass as bass
import concourse.mybir as mybir
from concourse import tile

# ---------------------------------------------------------------------------
# The walrus build in this container only supports ONE sync-wait per
# instruction; current Tile emits multi-wait instructions. Split the extra
# waits into single-wait NoOps on the same engine (engines execute their
# stream in order, so semantics are identical).
_orig_compile_bir = _bu.compile_bir_kernel


def _split_multiwaits(bir_bytes):
    d = json.loads(bir_bytes)
    n = 0
    for fn in d["functions"]:
        for blk in fn["blocks"]:
            out = []
            for ins in blk["instructions"]:
                si = ins.get("sync_info")
                waits = (si or {}).get("on_wait") or []
                if len(waits) > 1:
                    for w in waits[:-1]:
                        n += 1
                        out.append(
                            {
                                "name": f"WSPL{n}-{ins['name']}",
                                "opcode": "NoOp",
                                "engine": ins["engine"],
                                "debug": ins.get("debug", 0),
                                "ins": [],
                                "outs": [],
                                "sync_info": {"on_wait": [w]},
                            }
                        )
                    si["on_wait"] = [waits[-1]]
                out.append(ins)
            blk["instructions"] = out
    return json.dumps(d).encode()


def _patched_compile_bir(bir_json, tmpdir, neff_name="file.neff"):
    return _orig_compile_bir(_split_multiwaits(bir_json), tmpdir, neff_name)


if getattr(_bu.compile_bir_kernel, "__name__", "") != "_patched_compile_bir":
    _bu.compile_bir_kernel = _patched_compile_bir
    _b2j.compile_bir_kernel = _patched_compile_bir
# ---------------------------------------------------------------------------

# nn_MinConv2dGRUCell: x (4,32,64,32,32), h0 (4,1,64,32,32), W (128,64,3,3),
# b (128,). out = (4,32,64,32,32), h_next = out[:, -1:].
# Sharding: 8 cores = B(4) x H-half(2). Per core: conv over its 16 rows
# (+1 halo row each side), then the minGRU scan h_s = a_s*h_{s-1} + bv_s with
# a = sigmoid(-gate-b_g), bv = (1-a)*g, g = max(hidden+b_h+0.5, sigmoid(hidden+b_h)).
B, S, C_IN, H, WD = 4, 32, 64, 32, 32
HID = 64
N_CORES = 8
HH = H // 2  # 16 rows per core
PX = HH * WD  # 512 pixels per core
HPX = PX // 2  # 256 = free size after (128,256) repack
BLK = 8  # timesteps per tensor_tensor_scan instruction
NBLK = S // BLK
SEG = BLK + 1  # +1 reset column per pixel-tuple

F32 = mybir.dt.float32
F32R = mybir.dt.float32r
BF16 = mybir.dt.bfloat16

_CACHE = {}


def _build_nc():
    nc = bass.Bass(trn_type="TRN2")

    xpad_d = nc.dram_tensor("xpad", (S, C_IN, 18, 34), BF16, kind="ExternalInput")
    # paired taps (ky,0)+(ky,1) stacked on K; leftover taps (ky,2) separate
    wtp_d = nc.dram_tensor("wtp", (2 * C_IN, 3, 2 * HID), BF16, kind="ExternalInput")
    wts_d = nc.dram_tensor("wts", (C_IN, 3, 2 * HID), BF16, kind="ExternalInput")
    h0_d = nc.dram_tensor("h0r", (HID, PX), F32, kind="ExternalInput")
    # bias columns: 0 = -b_gate (rows 0:64), 1 = b_hid, 2 = b_hid+0.5
    # (rows 64:128), 3 = +b_gate (rows 0:64)
    bias_d = nc.dram_tensor("biases", (128, 4), F32, kind="ExternalInput")
    # raw scan-layout output: [block, channel, px*SEG] — host strips reset
    # columns and reorders; keeps the device-side DMA fully contiguous.
    out_d = nc.dram_tensor("out", (NBLK, HID, PX * SEG), F32, kind="ExternalOutput")

    AL = mybir.AluOpType
    ACTF = mybir.ActivationFunctionType

    with tile.TileContext(nc) as tc:
        with (
            tc.tile_pool(name="const", bufs=1) as constp,
            tc.tile_pool(name="xin", bufs=3) as xinp,
            tc.tile_pool(name="psum", bufs=6, space="PSUM") as psump,
            tc.tile_pool(name="ew", bufs=3) as ewp,
            tc.tile_pool(name="scan", bufs=2) as scanp,
            tc.tile_pool(name="outb", bufs=2) as outp,
        ):
            wtp_t = constp.tile([2 * C_IN, 3 * 2 * HID], BF16)
            nc.sync.dma_start(wtp_t[:], wtp_d[:, :, :])
            wts_t = constp.tile([C_IN, 3 * 2 * HID], BF16)
            nc.sync.dma_start(wts_t[:], wts_d[:, :, :])
            bias_t = constp.tile([128, 4], F32)
            nc.sync.dma_start(bias_t[:], bias_d[:, :])
            h0_t = constp.tile([HID, PX], F32)
            nc.sync.dma_start(h0_t[:], h0_d[:, :])

            o_prev = None
            for bs in range(NBLK):
                a_t = scanp.tile([HID, PX * SEG], F32, tag="a")
                bv_t = scanp.tile([HID, PX * SEG], F32, tag="bv")
                a3 = a_t[:].rearrange("p (px j) -> p px j", j=SEG)
                bv3 = bv_t[:].rearrange("p (px j) -> p px j", j=SEG)

                # reset columns: a=0, bv=h_init  =>  state restarts at h_init
                nc.gpsimd.memset(a3[:, :, 0], 0.0)
                if bs == 0:
                    nc.vector.tensor_copy(bv3[:, :, 0], h0_t[:])
                else:
                    op3 = o_prev[:].rearrange("p (px j) -> p px j", j=SEG)
                    nc.vector.tensor_copy(bv3[:, :, 0], op3[:, :, SEG - 1])

                for j in range(BLK):
                    s = bs * BLK + j
                    # x tile: partitions 0:64 = xpad, 64:128 = xpad shifted
                    # left by one column (tap kx+1 when read at kx)
                    x_t = xinp.tile([2 * C_IN, 18 * 34], BF16)
                    x3 = x_t[:].rearrange("p (r c) -> p r c", r=18)
                    xflat = xpad_d[s, :, :, :].rearrange("c r w -> c (r w)")
                    nc.sync.dma_start(x3[0:C_IN, :, :], xpad_d[s, :, :, :])
                    # shifted copy: bottom[f] = xpad[f+1]; the row-boundary
                    # bleed lands in pad col 33, which no tap reads
                    nc.sync.dma_start(x_t[C_IN:, 0:611], xflat[:, 1:612])

                    p_t = psump.tile([128, PX], F32)
                    for g in range(3):
                        # taps (g,0)+(g,1) in one K=128 matmul
                        nc.tensor.matmul(
                            p_t[:],
                            wtp_t[:, g * 128 : (g + 1) * 128],
                            x3[:, g : g + HH, 0:WD],
                            start=(g == 0),
                            stop=False,
                        )
                    for g in range(3):
                        # leftover tap (g,2), K=64
                        nc.tensor.matmul(
                            p_t[:],
                            wts_t[:, g * 128 : (g + 1) * 128],
                            x3[0:C_IN, g : g + HH, 2 : 2 + WD],
                            start=False,
                            stop=(g == 2),
                        )

                    # a = sigmoid(-gate - b_g)   [partitions 0:64]
                    nc.scalar.activation(
                        a3[:, :, 1 + j],
                        p_t[0:HID, :],
                        ACTF.Sigmoid,
                        bias=bias_t[0:HID, 0:1],
                        scale=-1.0,
                    )
                    # sig_h = sigmoid(hidden + b_h)   [partitions 64:128]
                    sh_t = ewp.tile([128, PX], F32, tag="sh")
                    nc.scalar.activation(
                        sh_t[HID:, :], p_t[HID:, :], ACTF.Sigmoid,
                        bias=bias_t[HID:, 1:2],
                    )
                    # g = max(hidden + b_h + 0.5, sig_h)   [partitions 64:128]
                    g_t = ewp.tile([128, PX], F32, tag="g")
                    nc.vector.scalar_tensor_tensor(
                        g_t[HID:, :], p_t[HID:, :], bias_t[HID:, 2:3], sh_t[HID:, :],
                        op0=AL.add, op1=AL.max,
                    )
                    # shift g down to the gate lanes
                    gl_t = ewp.tile([HID, PX], F32, tag="gl")
                    nc.gpsimd.dma_start(gl_t[:], g_t[HID:, :])
                    # z = 1 - a: alternate engines to balance ACT vs DVE load
                    z_t = ewp.tile([HID, PX], F32, tag="z")
                    if s % 2 == 0:
                        nc.scalar.activation(
                            z_t[:], p_t[0:HID, :], ACTF.Sigmoid,
                            bias=bias_t[0:HID, 3:4],
                        )
                    else:
                        nc.vector.tensor_scalar(
                            z_t[:], a3[:, :, 1 + j], -1.0, 1.0,
                            op0=AL.mult, op1=AL.add,
                        )
                    # bv = z * g   (Pool engine, frees DVE for the scan)
                    nc.gpsimd.tensor_tensor(bv3[:, :, 1 + j], z_t[:], gl_t[:], op=AL.mult)

                o_t = outp.tile([HID, PX * SEG], F32, tag="o")
                # h = a*h + bv along the segmented free dim
                nc.vector.tensor_tensor_scan(
                    o_t[:], a_t[:], bv_t[:], 0.0, op0=AL.mult, op1=AL.add
                )
                nc.sync.dma_start(out_d[bs, :, :], o_t[:])
                o_prev = o_t

    return nc


def kernel(x, h0, W, b):
    x = np.ascontiguousarray(x, dtype=np.float32)
    import ml_dtypes
    h0 = np.ascontiguousarray(h0, dtype=np.float32)
    W = np.ascontiguousarray(W, dtype=np.float32)
    b = np.ascontiguousarray(b, dtype=np.float32)

    if "nc" not in _CACHE:
        _CACHE["nc"] = _build_nc()
    nc = _CACHE["nc"]

    # host-side prep shared across cores
    wt = W.transpose(1, 2, 3, 0).reshape(C_IN, 9, 2 * HID)  # (ic, ky*3+kx, oc)
    wtp = np.ascontiguousarray(
        np.concatenate([wt[:, [0, 3, 6], :], wt[:, [1, 4, 7], :]], axis=0)
    ).astype(ml_dtypes.bfloat16)  # (128, 3, 128)
    wts = np.ascontiguousarray(wt[:, [2, 5, 8], :]).astype(ml_dtypes.bfloat16)
    biases = np.zeros((128, 4), dtype=np.float32)
    biases[0:HID, 0] = -b[0:HID]
    biases[HID:, 1] = b[HID:]
    biases[HID:, 2] = b[HID:] + 0.5
    biases[0:HID, 3] = b[0:HID]

    in_maps = []
    for core in range(N_CORES):
        bi, hh = core // 2, core % 2
        xpad = np.zeros((S, C_IN, 18, 34), dtype=ml_dtypes.bfloat16)
        r0 = hh * HH - 1  # global row of xpad row 0
        lo, hi = max(r0, 0), min(r0 + 18, H)
        xpad[:, :, lo - r0 : hi - r0, 1:33] = x[bi, :, :, lo:hi, :]
        h0r = np.ascontiguousarray(
            h0[bi, 0, :, hh * HH : (hh + 1) * HH, :].reshape(HID, PX)
        )
        in_maps.append(
            {"xpad": xpad, "wtp": wtp, "wts": wts, "h0r": h0r, "biases": biases}
        )

    res = _bu.run_bass_kernel_spmd(nc, in_maps, core_ids=list(range(N_CORES)))

    out = np.empty((B, S, HID, H, WD), dtype=np.float32)
    for core in range(N_CORES):
        bi, hh = core // 2, core % 2
        raw = res.results[core]["out"].reshape(NBLK, HID, PX, SEG)[:, :, :, 1:]
        # (blk, c, px, j) -> (s=blk*BLK+j, c, y, x)
        core_out = raw.transpose(0, 3, 1, 2).reshape(S, HID, HH, WD)
        out[bi, :, :, hh * HH : (hh + 1) * HH, :] = core_out
    h_next = out[:, -1:].copy()
    return out, h_next


# revision 23
# speedup vs baseline: 1.3210x; 1.1016x over previous
import json
import sys

sys.path.insert(0, "/opt/trn_rl_repo")

import numpy as np

import concourse.bass_utils as _bu
import concourse.bass2jax as _b2j
import concourse.bass as bass
import concourse.mybir as mybir
from concourse import tile

# ---------------------------------------------------------------------------
# The walrus build in this container only supports ONE sync-wait per
# instruction; current Tile emits multi-wait instructions. Split the extra
# waits into single-wait NoOps on the same engine (engines execute their
# stream in order, so semantics are identical).
_orig_compile_bir = _bu.compile_bir_kernel


def _split_multiwaits(bir_bytes):
    d = json.loads(bir_bytes)
    n = 0
    for fn in d["functions"]:
        for blk in fn["blocks"]:
            out = []
            for ins in blk["instructions"]:
                si = ins.get("sync_info")
                waits = (si or {}).get("on_wait") or []
                if len(waits) > 1:
                    for w in waits[:-1]:
                        n += 1
                        out.append(
                            {
                                "name": f"WSPL{n}-{ins['name']}",
                                "opcode": "NoOp",
                                "engine": ins["engine"],
                                "debug": ins.get("debug", 0),
                                "ins": [],
                                "outs": [],
                                "sync_info": {"on_wait": [w]},
                            }
                        )
                    si["on_wait"] = [waits[-1]]
                out.append(ins)
            blk["instructions"] = out
    return json.dumps(d).encode()


def _patched_compile_bir(bir_json, tmpdir, neff_name="file.neff"):
    return _orig_compile_bir(_split_multiwaits(bir_json), tmpdir, neff_name)


if getattr(_bu.compile_bir_kernel, "__name__", "") != "_patched_compile_bir":
    _bu.compile_bir_kernel = _patched_compile_bir
    _b2j.compile_bir_kernel = _patched_compile_bir
# ---------------------------------------------------------------------------

# nn_MinConv2dGRUCell: x (4,32,64,32,32), h0 (4,1,64,32,32), W (128,64,3,3),
# b (128,). out = (4,32,64,32,32), h_next = out[:, -1:].
# Sharding: 8 cores = B(4) x H-half(2). Per core: conv over its 16 rows
# (+1 halo row each side), then the minGRU scan h_s = a_s*h_{s-1} + bv_s with
# a = sigmoid(-gate-b_g), bv = (1-a)*g, g = max(hidden+b_h+0.5, sigmoid(hidden+b_h)).
B, S, C_IN, H, WD = 4, 32, 64, 32, 32
HID = 64
N_CORES = 8
HH = H // 2  # 16 rows per core
PX = HH * WD  # 512 pixels per core
HPX = PX // 2  # 256 = free size after (128,256) repack
BLK = 8  # timesteps per tensor_tensor_scan instruction
NBLK = S // BLK
SEG = BLK + 1  # +1 reset column per pixel-tuple

F32 = mybir.dt.float32
F32R = mybir.dt.float32r
BF16 = mybir.dt.bfloat16

_CACHE = {}


def _build_nc():
    nc = bass.Bass(trn_type="TRN2")

    xpad_d = nc.dram_tensor("xpad", (S, C_IN, 18, 34), BF16, kind="ExternalInput")
    # paired taps (ky,0)+(ky,1) stacked on K; leftover taps (ky,2) separate
    wtp_d = nc.dram_tensor("wtp", (2 * C_IN, 3, 2 * HID), BF16, kind="ExternalInput")
    wts_d = nc.dram_tensor("wts", (C_IN, 3, 2 * HID), BF16, kind="ExternalInput")
    h0_d = nc.dram_tensor("h0r", (HID, PX), F32, kind="ExternalInput")
    # bias columns: 0 = -b_gate (rows 0:64), 1 = b_hid, 2 = b_hid+0.5
    # (rows 64:128), 3 = +b_gate (rows 0:64)
    bias_d = nc.dram_tensor("biases", (128, 4), F32, kind="ExternalInput")
    # raw scan-layout output: [block, channel, px*SEG] — host strips reset
    # columns and reorders; keeps the device-side DMA fully contiguous.
    out_d = nc.dram_tensor("out", (NBLK, HID, PX * SEG), F32, kind="ExternalOutput")

    AL = mybir.AluOpType
    ACTF = mybir.ActivationFunctionType

    with tile.TileContext(nc) as tc:
        with (
            tc.tile_pool(name="const", bufs=1) as constp,
            tc.tile_pool(name="xin", bufs=4) as xinp,
            tc.tile_pool(name="psum", bufs=8, space="PSUM") as psump,
            tc.tile_pool(name="ew", bufs=4) as ewp,
            tc.tile_pool(name="scan", bufs=2) as scanp,
            tc.tile_pool(name="outb", bufs=2) as outp,
        ):
            wtp_t = constp.tile([2 * C_IN, 3 * 2 * HID], BF16)
            nc.sync.dma_start(wtp_t[:], wtp_d[:, :, :])
            wts_t = constp.tile([C_IN, 3 * 2 * HID], BF16)
            nc.sync.dma_start(wts_t[:], wts_d[:, :, :])
            bias_t = constp.tile([128, 4], F32)
            nc.sync.dma_start(bias_t[:], bias_d[:, :])
            h0_t = constp.tile([HID, PX], F32)
            nc.sync.dma_start(h0_t[:], h0_d[:, :])

            o_prev = None
            for bs in range(NBLK):
                a_t = scanp.tile([HID, PX * SEG], F32, tag="a")
                bv_t = scanp.tile([HID, PX * SEG], F32, tag="bv")
                a3 = a_t[:].rearrange("p (px j) -> p px j", j=SEG)
                bv3 = bv_t[:].rearrange("p (px j) -> p px j", j=SEG)

                # reset columns: a=0, bv=h_init  =>  state restarts at h_init
                nc.gpsimd.memset(a3[:, :, 0], 0.0)
                if bs == 0:
                    nc.vector.tensor_copy(bv3[:, :, 0], h0_t[:])
                else:
                    op3 = o_prev[:].rearrange("p (px j) -> p px j", j=SEG)
                    nc.vector.tensor_copy(bv3[:, :, 0], op3[:, :, SEG - 1])

                for j in range(BLK):
                    s = bs * BLK + j
                    # x tile: partitions 0:64 = xpad, 64:128 = xpad shifted
                    # left by one column (tap kx+1 when read at kx)
                    x_t = xinp.tile([2 * C_IN, 18 * 34], BF16)
                    x3 = x_t[:].rearrange("p (r c) -> p r c", r=18)
                    xflat = xpad_d[s, :, :, :].rearrange("c r w -> c (r w)")
                    nc.sync.dma_start(x3[0:C_IN, :, :], xpad_d[s, :, :, :])
                    # shifted copy: bottom[f] = xpad[f+1]; the row-boundary
                    # bleed lands in pad col 33, which no tap reads
                    nc.sync.dma_start(x_t[C_IN:, 0:611], xflat[:, 1:612])

                    p_t = psump.tile([128, PX], F32)
                    for g in range(3):
                        # taps (g,0)+(g,1) in one K=128 matmul
                        nc.tensor.matmul(
                            p_t[:],
                            wtp_t[:, g * 128 : (g + 1) * 128],
                            x3[:, g : g + HH, 0:WD],
                            start=(g == 0),
                            stop=False,
                        )
                    for g in range(3):
                        # leftover tap (g,2), K=64
                        nc.tensor.matmul(
                            p_t[:],
                            wts_t[:, g * 128 : (g + 1) * 128],
                            x3[0:C_IN, g : g + HH, 2 : 2 + WD],
                            start=False,
                            stop=(g == 2),
                        )

                    # a = sigmoid(-gate - b_g)   [partitions 0:64]
                    nc.scalar.activation(
                        a3[:, :, 1 + j],
                        p_t[0:HID, :],
                        ACTF.Sigmoid,
                        bias=bias_t[0:HID, 0:1],
                        scale=-1.0,
                    )
                    # sig_h = sigmoid(hidden + b_h)   [partitions 64:128]
                    sh_t = ewp.tile([128, PX], F32, tag="sh")
                    nc.scalar.activation(
                        sh_t[HID:, :], p_t[HID:, :], ACTF.Sigmoid,
                        bias=bias_t[HID:, 1:2],
                    )
                    # g = max(hidden + b_h + 0.5, sig_h)   [partitions 64:128]
                    g_t = ewp.tile([128, PX], F32, tag="g")
                    nc.vector.scalar_tensor_tensor(
                        g_t[HID:, :], p_t[HID:, :], bias_t[HID:, 2:3], sh_t[HID:, :],
                        op0=AL.add, op1=AL.max,
                    )
                    # shift g down to the gate lanes
                    gl_t = ewp.tile([HID, PX], F32, tag="gl")
                    nc.gpsimd.dma_start(gl_t[:], g_t[HID:, :])
                    # z = 1 - a: alternate engines to balance ACT vs DVE load
                    z_t = ewp.tile([HID, PX], F32, tag="z")
                    nc.scalar.activation(
                        z_t[:], p_t[0:HID, :], ACTF.Sigmoid,
                        bias=bias_t[0:HID, 3:4],
                    )
                    # bv = z * g   (Pool engine, frees DVE for the scan)
                    nc.gpsimd.tensor_tensor(bv3[:, :, 1 + j], z_t[:], gl_t[:], op=AL.mult)

                o_t = outp.tile([HID, PX * SEG], F32, tag="o")
                # h = a*h + bv along the segmented free dim
                nc.vector.tensor_tensor_scan(
                    o_t[:], a_t[:], bv_t[:], 0.0, op0=AL.mult, op1=AL.add
                )
                nc.sync.dma_start(out_d[bs, :, :], o_t[:])
                o_prev = o_t

    return nc


def kernel(x, h0, W, b):
    x = np.ascontiguousarray(x, dtype=np.float32)
    import ml_dtypes
    h0 = np.ascontiguousarray(h0, dtype=np.float32)
    W = np.ascontiguousarray(W, dtype=np.float32)
    b = np.ascontiguousarray(b, dtype=np.float32)

    if "nc" not in _CACHE:
        _CACHE["nc"] = _build_nc()
    nc = _CACHE["nc"]

    # host-side prep shared across cores
    wt = W.transpose(1, 2, 3, 0).reshape(C_IN, 9, 2 * HID)  # (ic, ky*3+kx, oc)
    wtp = np.ascontiguousarray(
        np.concatenate([wt[:, [0, 3, 6], :], wt[:, [1, 4, 7], :]], axis=0)
    ).astype(ml_dtypes.bfloat16)  # (128, 3, 128)
    wts = np.ascontiguousarray(wt[:, [2, 5, 8], :]).astype(ml_dtypes.bfloat16)
    biases = np.zeros((128, 4), dtype=np.float32)
    biases[0:HID, 0] = -b[0:HID]
    biases[HID:, 1] = b[HID:]
    biases[HID:, 2] = b[HID:] + 0.5
    biases[0:HID, 3] = b[0:HID]

    in_maps = []
    for core in range(N_CORES):
        bi, hh = core // 2, core % 2
        xpad = np.zeros((S, C_IN, 18, 34), dtype=ml_dtypes.bfloat16)
        r0 = hh * HH - 1  # global row of xpad row 0
        lo, hi = max(r0, 0), min(r0 + 18, H)
        xpad[:, :, lo - r0 : hi - r0, 1:33] = x[bi, :, :, lo:hi, :]
        h0r = np.ascontiguousarray(
            h0[bi, 0, :, hh * HH : (hh + 1) * HH, :].reshape(HID, PX)
        )
        in_maps.append(
            {"xpad": xpad, "wtp": wtp, "wts": wts, "h0r": h0r, "biases": biases}
        )

    res = _bu.run_bass_kernel_spmd(nc, in_maps, core_ids=list(range(N_CORES)))

    out = np.empty((B, S, HID, H, WD), dtype=np.float32)
    for core in range(N_CORES):
        bi, hh = core // 2, core % 2
        raw = res.results[core]["out"].reshape(NBLK, HID, PX, SEG)[:, :, :, 1:]
        # (blk, c, px, j) -> (s=blk*BLK+j, c, y, x)
        core_out = raw.transpose(0, 3, 1, 2).reshape(S, HID, HH, WD)
        out[bi, :, :, hh * HH : (hh + 1) * HH, :] = core_out
    h_next = out[:, -1:].copy()
    return out, h_next


# revision 24
# speedup vs baseline: 1.3551x; 1.0258x over previous
import json
import sys

sys.path.insert(0, "/opt/trn_rl_repo")

import numpy as np

import concourse.bass_utils as _bu
import concourse.bass2jax as _b2j
import concourse.bass as bass
import concourse.mybir as mybir
from concourse import tile

# ---------------------------------------------------------------------------
# The walrus build in this container only supports ONE sync-wait per
# instruction; current Tile emits multi-wait instructions. Split the extra
# waits into single-wait NoOps on the same engine (engines execute their
# stream in order, so semantics are identical).
_orig_compile_bir = _bu.compile_bir_kernel


def _split_multiwaits(bir_bytes):
    d = json.loads(bir_bytes)
    n = 0
    for fn in d["functions"]:
        for blk in fn["blocks"]:
            out = []
            for ins in blk["instructions"]:
                si = ins.get("sync_info")
                waits = (si or {}).get("on_wait") or []
                if len(waits) > 1:
                    for w in waits[:-1]:
                        n += 1
                        out.append(
                            {
                                "name": f"WSPL{n}-{ins['name']}",
                                "opcode": "NoOp",
                                "engine": ins["engine"],
                                "debug": ins.get("debug", 0),
                                "ins": [],
                                "outs": [],
                                "sync_info": {"on_wait": [w]},
                            }
                        )
                    si["on_wait"] = [waits[-1]]
                out.append(ins)
            blk["instructions"] = out
    return json.dumps(d).encode()


def _patched_compile_bir(bir_json, tmpdir, neff_name="file.neff"):
    return _orig_compile_bir(_split_multiwaits(bir_json), tmpdir, neff_name)


if getattr(_bu.compile_bir_kernel, "__name__", "") != "_patched_compile_bir":
    _bu.compile_bir_kernel = _patched_compile_bir
    _b2j.compile_bir_kernel = _patched_compile_bir
# ---------------------------------------------------------------------------

# nn_MinConv2dGRUCell: x (4,32,64,32,32), h0 (4,1,64,32,32), W (128,64,3,3),
# b (128,). out = (4,32,64,32,32), h_next = out[:, -1:].
# Sharding: 8 cores = B(4) x H-half(2). Per core: conv over its 16 rows
# (+1 halo row each side), then the minGRU scan h_s = a_s*h_{s-1} + bv_s with
# a = sigmoid(-gate-b_g), bv = (1-a)*g, g = max(hidden+b_h+0.5, sigmoid(hidden+b_h)).
B, S, C_IN, H, WD = 4, 32, 64, 32, 32
HID = 64
N_CORES = 8
HH = H // 2  # 16 rows per core
PX = HH * WD  # 512 pixels per core
HPX = PX // 2  # 256 = free size after (128,256) repack
BLK = 8  # timesteps per tensor_tensor_scan instruction
NBLK = S // BLK
SEG = BLK + 1  # +1 reset column per pixel-tuple

F32 = mybir.dt.float32
F32R = mybir.dt.float32r
BF16 = mybir.dt.bfloat16

_CACHE = {}


def _build_nc():
    nc = bass.Bass(trn_type="TRN2")

    xpad_d = nc.dram_tensor("xpad", (S, C_IN, 18, 34), BF16, kind="ExternalInput")
    # paired taps (ky,0)+(ky,1) stacked on K; leftover taps (ky,2) separate
    wtp_d = nc.dram_tensor("wtp", (2 * C_IN, 3, 2 * HID), BF16, kind="ExternalInput")
    wts_d = nc.dram_tensor("wts", (C_IN, 3, 2 * HID), BF16, kind="ExternalInput")
    h0_d = nc.dram_tensor("h0r", (HID, PX), F32, kind="ExternalInput")
    # bias columns: 0 = -b_gate (rows 0:64), 1 = b_hid, 2 = b_hid+0.5
    # (rows 64:128), 3 = +b_gate (rows 0:64)
    bias_d = nc.dram_tensor("biases", (128, 4), F32, kind="ExternalInput")
    # raw scan-layout output: [block, channel, px*SEG] — host strips reset
    # columns and reorders; keeps the device-side DMA fully contiguous.
    out_d = nc.dram_tensor("out", (NBLK, HID, PX * SEG), F32, kind="ExternalOutput")

    AL = mybir.AluOpType
    ACTF = mybir.ActivationFunctionType

    with tile.TileContext(nc) as tc:
        with (
            tc.tile_pool(name="const", bufs=1) as constp,
            tc.tile_pool(name="xin", bufs=4) as xinp,
            tc.tile_pool(name="psum", bufs=8, space="PSUM") as psump,
            tc.tile_pool(name="ew", bufs=4) as ewp,
            tc.tile_pool(name="scan", bufs=2) as scanp,
            tc.tile_pool(name="outb", bufs=2) as outp,
        ):
            wtp_t = constp.tile([2 * C_IN, 3 * 2 * HID], BF16)
            nc.sync.dma_start(wtp_t[:], wtp_d[:, :, :])
            wts_t = constp.tile([C_IN, 3 * 2 * HID], BF16)
            nc.sync.dma_start(wts_t[:], wts_d[:, :, :])
            bias_t = constp.tile([128, 4], F32)
            nc.sync.dma_start(bias_t[:], bias_d[:, :])
            h0_t = constp.tile([HID, PX], F32)
            nc.sync.dma_start(h0_t[:], h0_d[:, :])

            o_prev = None
            for bs in range(NBLK):
                a_t = scanp.tile([HID, PX * SEG], F32, tag="a")
                bv_t = scanp.tile([HID, PX * SEG], F32, tag="bv")
                a3 = a_t[:].rearrange("p (px j) -> p px j", j=SEG)
                bv3 = bv_t[:].rearrange("p (px j) -> p px j", j=SEG)

                # reset columns: a=0, bv=h_init  =>  state restarts at h_init
                nc.gpsimd.memset(a3[:, :, 0], 0.0)
                if bs == 0:
                    nc.vector.tensor_copy(bv3[:, :, 0], h0_t[:])
                else:
                    op3 = o_prev[:].rearrange("p (px j) -> p px j", j=SEG)
                    nc.vector.tensor_copy(bv3[:, :, 0], op3[:, :, SEG - 1])

                for j in range(BLK):
                    s = bs * BLK + j
                    # x tile: partitions 0:64 = xpad, 64:128 = xpad shifted
                    # left by one column (tap kx+1 when read at kx)
                    x_t = xinp.tile([2 * C_IN, 18 * 34], BF16)
                    x3 = x_t[:].rearrange("p (r c) -> p r c", r=18)
                    xflat = xpad_d[s, :, :, :].rearrange("c r w -> c (r w)")
                    nc.sync.dma_start(x3[0:C_IN, :, :], xpad_d[s, :, :, :])
                    # shifted copy: bottom[f] = xpad[f+1]; the row-boundary
                    # bleed lands in pad col 33, which no tap reads
                    nc.sync.dma_start(x_t[C_IN:, 0:611], xflat[:, 1:612])

                    p_t = psump.tile([128, PX], F32)
                    for g in range(3):
                        # taps (g,0)+(g,1) in one K=128 matmul
                        nc.tensor.matmul(
                            p_t[:],
                            wtp_t[:, g * 128 : (g + 1) * 128],
                            x3[:, g : g + HH, 0:WD],
                            start=(g == 0),
                            stop=False,
                        )
                    for g in range(3):
                        # leftover tap (g,2), K=64
                        nc.tensor.matmul(
                            p_t[:],
                            wts_t[:, g * 128 : (g + 1) * 128],
                            x3[0:C_IN, g : g + HH, 2 : 2 + WD],
                            start=False,
                            stop=(g == 2),
                        )

                    # a = sigmoid(-gate - b_g)   [partitions 0:64]
                    nc.scalar.activation(
                        a3[:, :, 1 + j],
                        p_t[0:HID, :],
                        ACTF.Sigmoid,
                        bias=bias_t[0:HID, 0:1],
                        scale=-1.0,
                    )
                    # sig_h = sigmoid(hidden + b_h)   [partitions 64:128]
                    sh_t = ewp.tile([128, PX], BF16, tag="sh")
                    nc.scalar.activation(
                        sh_t[HID:, :], p_t[HID:, :], ACTF.Sigmoid,
                        bias=bias_t[HID:, 1:2],
                    )
                    # g = max(hidden + b_h + 0.5, sig_h)   [partitions 64:128]
                    g_t = ewp.tile([128, PX], BF16, tag="g")
                    nc.vector.scalar_tensor_tensor(
                        g_t[HID:, :], p_t[HID:, :], bias_t[HID:, 2:3], sh_t[HID:, :],
                        op0=AL.add, op1=AL.max,
                    )
                    # shift g down to the gate lanes
                    gl_t = ewp.tile([HID, PX], BF16, tag="gl")
                    nc.sync.dma_start(gl_t[:], g_t[HID:, :])
                    # z = 1 - a: alternate engines to balance ACT vs DVE load
                    z_t = ewp.tile([HID, PX], BF16, tag="z")
                    nc.scalar.activation(
                        z_t[:], p_t[0:HID, :], ACTF.Sigmoid,
                        bias=bias_t[0:HID, 3:4],
                    )
                    # bv = z * g   (Pool engine, frees DVE for the scan)
                    nc.gpsimd.tensor_tensor(bv3[:, :, 1 + j], z_t[:], gl_t[:], op=AL.mult)

                o_t = outp.tile([HID, PX * SEG], F32, tag="o")
                # h = a*h + bv along the segmented free dim
                nc.vector.tensor_tensor_scan(
                    o_t[:], a_t[:], bv_t[:], 0.0, op0=AL.mult, op1=AL.add
                )
                nc.sync.dma_start(out_d[bs, :, :], o_t[:])
                o_prev = o_t

    return nc


def kernel(x, h0, W, b):
    x = np.ascontiguousarray(x, dtype=np.float32)
    import ml_dtypes
    h0 = np.ascontiguousarray(h0, dtype=np.float32)
    W = np.ascontiguousarray(W, dtype=np.float32)
    b = np.ascontiguousarray(b, dtype=np.float32)

    if "nc" not in _CACHE:
        _CACHE["nc"] = _build_nc()
    nc = _CACHE["nc"]

    # host-side prep shared across cores
    wt = W.transpose(1, 2, 3, 0).reshape(C_IN, 9, 2 * HID)  # (ic, ky*3+kx, oc)
    wtp = np.ascontiguousarray(
        np.concatenate([wt[:, [0, 3, 6], :], wt[:, [1, 4, 7], :]], axis=0)
    ).astype(ml_dtypes.bfloat16)  # (128, 3, 128)
    wts = np.ascontiguousarray(wt[:, [2, 5, 8], :]).astype(ml_dtypes.bfloat16)
    biases = np.zeros((128, 4), dtype=np.float32)
    biases[0:HID, 0] = -b[0:HID]
    biases[HID:, 1] = b[HID:]
    biases[HID:, 2] = b[HID:] + 0.5
    biases[0:HID, 3] = b[0:HID]

    in_maps = []
    for core in range(N_CORES):
        bi, hh = core // 2, core % 2
        xpad = np.zeros((S, C_IN, 18, 34), dtype=ml_dtypes.bfloat16)
        r0 = hh * HH - 1  # global row of xpad row 0
        lo, hi = max(r0, 0), min(r0 + 18, H)
        xpad[:, :, lo - r0 : hi - r0, 1:33] = x[bi, :, :, lo:hi, :]
        h0r = np.ascontiguousarray(
            h0[bi, 0, :, hh * HH : (hh + 1) * HH, :].reshape(HID, PX)
        )
        in_maps.append(
            {"xpad": xpad, "wtp": wtp, "wts": wts, "h0r": h0r, "biases": biases}
        )

    res = _bu.run_bass_kernel_spmd(nc, in_maps, core_ids=list(range(N_CORES)))

    out = np.empty((B, S, HID, H, WD), dtype=np.float32)
    for core in range(N_CORES):
        bi, hh = core // 2, core % 2
        raw = res.results[core]["out"].reshape(NBLK, HID, PX, SEG)[:, :, :, 1:]
        # (blk, c, px, j) -> (s=blk*BLK+j, c, y, x)
        core_out = raw.transpose(0, 3, 1, 2).reshape(S, HID, HH, WD)
        out[bi, :, :, hh * HH : (hh + 1) * HH, :] = core_out
    h_next = out[:, -1:].copy()
    return out, h_next
